# revision 45
# baseline (speedup 1.0000x reference)
"""Trainium2 Bass kernel for nn_Attention (B=4, S=1024, D=1024, H=16).

Sharding: 8 cores = 4 batches x 2 query-halves. Core i handles batch i//2,
query rows [(i%2)*512, (i%2)*512+512). Each core computes the full K/V
projections for its batch (duplicated across the 2 cores sharing a batch),
all 16 heads of attention for its query slice, and the output projection.
No collectives; the output is gathered host-side.

Device dataflow (per core) — fp16 matmul operands, fp32 PSUM accumulation:
  - host passes pre-transposed qT [D,SQ], kT/vT [D,SK], W{v,o}.T [D,D], and
    m-blocked W{k,q}.T [NT,D,128] (PE contracts over the partition dim, so
    both matmul operands need the contraction dim on partitions; transposing
    and blocking on host costs nothing on HW)
  - qhT[o,sq] = (Wq.T*SCALE).T-tiles @ qT     (o on partitions)
  - khT[o,sk] likewise; vh[sk, h, dh] natural via vT-as-stationary
  - scoresT[sk,sq] per head = khT-tile.T @ qhT; the two heads of a pair run
    as K=64 matmuls packed at PE row strips 0:64 / 64:128 (concurrent on
    disjoint row groups), writing the two halves of one [128, 2*SQ] PSUM
    tile -> ONE fused exp per pair
  - expT = exp(scoresT) on ACT (no max subtraction: |scores| < ~4 here,
    and softmax(x) == softmax(x - max) exactly)
  - ctxT_aug[dh+1, sq] += [vh | 1].T @ expT  (ones column makes row 64 the
    softmax denominator, riding free on the ctx matmul)
  - ctx PSUM is drained fast on DVE (sum-row copy + approx-reciprocal
    first, ctx rows after); the gpsimd broadcast + normalization multiply
    run later, off the critical path (the last pair broadcasts via a fp16
    ones-row matmul on the PE instead, so the output projection can start)
  - out[sq,o] = ctxT-tiles.T @ Wo.T (+bo via a K=1 ones-row matmul) in fp16

Scheduling (the trace-driven part):
  - input DMAs are spread across 4 engine queues in first-use order so the
    ramp is limited by HBM bandwidth, not queue serialization
  - the PE warms on c-groups (q-projection, gated only on qT+wq) instead of
    dummy matmuls during the DMA ramp
  - projection "filler" groups are drained into the attention loop with
    TARGETED just-in-time drains plus a slow generic pace (1 group per 2
    steps), so filler work survives into pairs 5-6 where exp latency would
    otherwise stall the PE (and let HAM re-throttle the clock)
  - during pair 7 the first two output-projection groups pre-accumulate
    kk=0..6; after the last ctx matmul the remaining six groups
    pre-accumulate on the PSUM banks just freed by the scores/ctx pools, so
    only eight kk=7 matmuls + drains remain after the final normalize
  - output is fp16 (host casts back), one [128,1024] DMA per row-block on
    rotating queues

Bias handling (exact): bq via ACT Identity-bias on the qh drain; bk dropped
(softmax is invariant to per-query score shifts); bv folded into bo on the
host (softmax rows sum to 1, so ctx gains +bv and out gains +Wo@bv); bo via
a K=1 ones-row matmul accumulated into the output-projection PSUM.
"""

import sys

import numpy as np

if "/opt/trn_rl_repo" not in sys.path:
    sys.path.insert(0, "/opt/trn_rl_repo")

B, S, D, H = 4, 1024, 1024, 16
HD = D // H                      # 64
SCALE = 1.0 / float(np.sqrt(HD))
N_CORES = 8
SQ = S // 2                      # 512 query rows per core
SK = S                           # full key length
P = 128
NT = D // P                      # 8 feature tiles
SKT = SK // P                    # 8 key tiles
NPAIR = H // 2                   # 8 head pairs
NC2 = 512                        # max matmul free dim (one PSUM bank)

_CACHE = {}


def _build_program():
    from contextlib import ExitStack

    import concourse.bass as bass
    import concourse.tile as tile
    from concourse import bacc, mybir

    F32 = mybir.dt.float32
    F16 = mybir.dt.float16
    AF = mybir.ActivationFunctionType

    nc = bacc.Bacc(
        "TRN2", target_bir_lowering=False, debug=False, num_devices=N_CORES
    )

    qT_d = nc.dram_tensor("qT", [P, NT, SQ], F16, kind="ExternalInput").ap()
    # kT blocked by key-half (b-group reads one half of every kk chunk);
    # vT blocked by key-block j (a-group reads one j-slice of every kk chunk)
    kT_d = nc.dram_tensor("kT", [2, P, NT, NC2], F16,
                          kind="ExternalInput").ap()
    vT_d = nc.dram_tensor("vT", [SKT, P, NT, P], F16,
                          kind="ExternalInput").ap()
    wqT_d = nc.dram_tensor("wqT", [NT, P, NT, P], F16,
                           kind="ExternalInput").ap()
    wkT_d = nc.dram_tensor("wkT", [NT, P, NT, P], F16,
                           kind="ExternalInput").ap()
    wvT_d = nc.dram_tensor("wvT", [2, P, NT, NC2], F16,
                           kind="ExternalInput").ap()
    woT_d = nc.dram_tensor("woT", [P, NT, D], F16, kind="ExternalInput").ap()
    bq_d = nc.dram_tensor("bq", [D], F32, kind="ExternalInput").ap()
    bo_d = nc.dram_tensor("bo", [D], F16, kind="ExternalInput").ap()
    out_d = nc.dram_tensor("out", [SQ, D], F16, kind="ExternalOutput").ap()

    mm = lambda *a, **k: nc.tensor.matmul(*a, **k)

    with tile.TileContext(nc) as tc, ExitStack() as ctx:
        persist = ctx.enter_context(tc.tile_pool(name="persist", bufs=1))
        epool = ctx.enter_context(tc.tile_pool(name="epool", bufs=4))
        rpool = ctx.enter_context(tc.tile_pool(name="rp", bufs=2))
        opool = ctx.enter_context(tc.tile_pool(name="outp", bufs=2))
        pp = ctx.enter_context(tc.tile_pool(name="pp", space="PSUM", bufs=2))
        pS = ctx.enter_context(tc.tile_pool(name="pS", space="PSUM", bufs=2))
        pX = ctx.enter_context(tc.tile_pool(name="pX", space="PSUM", bufs=1))

        # persistent data tiles
        qT_sb = persist.tile([P, NT, SQ], F16)
        kT_sb = persist.tile([P, NT, SK], F16)
        vT_sb = persist.tile([P, NT, SK], F16)
        wq = persist.tile([P, NT, D], F16)
        wk = persist.tile([P, NT, D], F16)
        wv = persist.tile([P, NT, D], F16)
        wo = persist.tile([P, NT, D], F16)
        qhT = persist.tile([P, NT, SQ], F16)        # [o'%128, o'//128, sq]
        khT = persist.tile([P, NT, SK], F16)
        vh = persist.tile([P, SKT, H, HD + 1], F16)  # [sk%128, sk//128, h, .]
        ctxT = persist.tile([P, NT, SQ], F16)
        ctxU = persist.tile([P, NT, SQ], F16)    # unnormalized ctx (drain)
        bq_sb = persist.tile([P, NT], F32)
        bo_row = persist.tile([1, D], F16)

        # ---- input DMAs: first-use priority, spread over 3 queues ----
        # descriptor generation costs ~1.5-3us of engine time per dma, so
        # each queue's issue order is also its engine-time budget: the
        # scalar queue carries only what gates pair 0, sync carries the
        # k-side, gpsimd the v-side
        # per-kk chunks so subtile deps release each contraction matmul as
        # its chunk lands, instead of gating pair 0 on whole-tensor DMAs
        nc.scalar.dma_start(qT_sb[:, 0, :], qT_d[:, 0, :])
        nc.scalar.dma_start(wq[:, :, 0:P], wqT_d[0])
        for kk in range(1, NT):
            nc.scalar.dma_start(qT_sb[:, kk, :], qT_d[:, kk, :])
        nc.scalar.dma_start(wq[:, :, P:2 * P], wqT_d[1])
        nc.scalar.dma_start(wv[:, :, 0:NC2], wvT_d[0])
        nc.scalar.dma_start(wq[:, :, 2 * P:3 * P], wqT_d[2])
        nc.scalar.dma_start(wq[:, :, 3 * P:4 * P], wqT_d[3])
        nc.sync.dma_start(kT_sb[:, :, 0:NC2], kT_d[0])
        nc.sync.dma_start(wk[:, :, 0:P], wkT_d[0])
        nc.sync.dma_start(kT_sb[:, :, NC2:SK], kT_d[1])
        for m in range(1, NT):
            nc.sync.dma_start(wk[:, :, m * P:(m + 1) * P], wkT_d[m])
        nc.gpsimd.dma_start(out=bq_sb, in_=bq_d.rearrange("(m p) -> p m", p=P))
        nc.gpsimd.dma_start(out=bo_row, in_=bo_d.rearrange("(o d) -> o d", o=1))
        for j in range(SKT):
            nc.gpsimd.dma_start(vT_sb[:, :, j * P:(j + 1) * P], vT_d[j])
        for m in range(4, NT):
            nc.sync.dma_start(wq[:, :, m * P:(m + 1) * P], wqT_d[m])
        nc.gpsimd.dma_start(wv[:, :, NC2:D], wvT_d[1])
        nc.sync.dma_start(wo, woT_d)
        for j in range(SKT):
            nc.vector.memset(vh[:, j, :, HD].bitcast(mybir.dt.uint16), 0x3C00)
        warm = rpool.tile([1, 1], F32, name="warm")
        nc.vector.memset(warm, 0.0)
        nc.scalar.activation(warm, warm, AF.Exp)
        # short dummy-matmul ramp on a zeroed tile: HAM sees a busy PE and
        # unthrottles to full clock just as the first real matmuls start
        wz = persist.tile([P, NC2], F16)
        nc.vector.memset(wz, 0.0)

        def pe_warm(n):
            psw = pp.tile([P, NC2], F32, name="ppt")
            for _ in range(n):
                mm(psw, wz[:, 0:P], wz, start=True, stop=True)

        pe_warm(12)
        ones_sb = persist.tile([1, P], F16)
        nc.vector.memset(ones_sb, 1.0)

        # ---- emit-group helpers (each = one 8-MM PSUM accumulation) ----
        def a_group(j, c):  # v-proj: vh[:, j, heads c*8..c*8+7]
            psa = pp.tile([P, NC2], F32, name="ppt")
            for kk in range(NT):
                mm(psa, vT_sb[:, kk, j * P:(j + 1) * P],
                   wv[:, kk, c * NC2:(c + 1) * NC2],
                   start=kk == 0, stop=kk == NT - 1)
            nc.vector.tensor_copy(
                vh[:, j, c * 8:(c + 1) * 8, 0:HD],
                psa.rearrange("p (h d) -> p h d", d=HD),
            )

        def b_group(m, c):  # k-proj: khT[:, m, c*512:...]
            psb = pp.tile([P, NC2], F32, name="ppt")
            for kk in range(NT):
                mm(psb, wk[:, kk, m * P:(m + 1) * P],
                   kT_sb[:, kk, c * NC2:(c + 1) * NC2],
                   start=kk == 0, stop=kk == NT - 1)
            nc.vector.tensor_copy(khT[:, m, c * NC2:(c + 1) * NC2], psb)

        def c_group(m):  # q-proj: qhT[:, m, :]
            psc = pp.tile([P, NC2], F32, name="ppt")
            for kk in range(NT):
                mm(psc, wq[:, kk, m * P:(m + 1) * P], qT_sb[:, kk, :],
                   start=kk == 0, stop=kk == NT - 1)
            nc.vector.tensor_scalar_add(qhT[:, m, :], psc, bq_sb[:, m:m + 1])

        # ---- ramp: only what pair 0 needs, so scores start ~25us ----
        c_group(0)
        b_group(0, 0)
        b_group(0, 1)
        c_group(1)

        # ---- filler stream with targeted + paced drains ----
        filler = []          # ordered list of (label, emit_fn)
        emitted = set()
        by_label = {}

        def drain_until(labels):
            # targeted: emit exactly the named groups (in the given order);
            # labels not in the filler set (e.g. ramp-emitted c0/c1) skip
            for lbl in labels:
                if lbl in by_label and lbl not in emitted:
                    emitted.add(lbl)
                    by_label[lbl]()

        def drain_next(n=1):
            done = 0
            for lbl, fn in filler:
                if lbl not in emitted:
                    emitted.add(lbl)
                    fn()
                    done += 1
                    if done >= n:
                        return

        def add_filler(lbl, fn):
            filler.append((lbl, fn))
            by_label[lbl] = fn

        add_filler("c2", lambda: c_group(2))
        add_filler("c3", lambda: c_group(3))
        add_filler("c4", lambda: c_group(4))
        add_filler("c5", lambda: c_group(5))
        for j in range(SKT):
            add_filler(f"a{j}c0", lambda j=j: a_group(j, 0))
        for m in (1, 2, 3):
            add_filler(f"b{m}a", lambda m=m: b_group(m, 0))
            add_filler(f"b{m}b", lambda m=m: b_group(m, 1))
        for j in range(4):
            add_filler(f"a{j}c1", lambda j=j: a_group(j, 1))
        add_filler("b4a", lambda: b_group(4, 0))
        add_filler("b4b", lambda: b_group(4, 1))
        for j in range(4, SKT):
            add_filler(f"a{j}c1", lambda j=j: a_group(j, 1))
        add_filler("b5a", lambda: b_group(5, 0))
        add_filler("b5b", lambda: b_group(5, 1))
        add_filler("c6", lambda: c_group(6))
        add_filler("b6a", lambda: b_group(6, 0))
        add_filler("b6b", lambda: b_group(6, 1))
        add_filler("c7", lambda: c_group(7))
        add_filler("b7a", lambda: b_group(7, 0))
        add_filler("b7b", lambda: b_group(7, 1))

        # ---- attention ----
        def scores(t, j):
            sp = pS.tile([P, 2, SQ], F32, name="sp")
            mm(sp[:, 0, :], khT[0:HD, t, j * P:(j + 1) * P], qhT[0:HD, t, :],
               start=True, stop=True)
            mm(sp[:, 1, :], khT[HD:P, t, j * P:(j + 1) * P], qhT[HD:P, t, :],
               start=True, stop=True)
            return sp

        def normalize(t, r0, r1):
            rb0 = rpool.tile([P, SQ], F32, name="rb0")
            rb1 = rpool.tile([P, SQ], F32, name="rb1")
            nc.gpsimd.partition_broadcast(rb0, r0)
            nc.gpsimd.partition_broadcast(rb1, r1)
            nc.vector.tensor_mul(ctxT[0:HD, t, :], ctxU[0:HD, t, :],
                                 rb0[0:HD, :])
            nc.vector.tensor_mul(ctxT[HD:P, t, :], ctxU[HD:P, t, :],
                                 rb1[HD:P, :])

        # output-projection groups: G[sqt][c], pre-accumulated kk=0..6 (+bo)
        # then finished with the kk=7 matmul once pair 7's ctxT lands
        psE = {}

        def e_pre(sqt, c, ps):
            psE[(sqt, c)] = ps
            for kk in range(NT - 1):
                mm(ps, ctxT[:, kk, sqt * P:(sqt + 1) * P],
                   wo[:, kk, c * NC2:(c + 1) * NC2],
                   start=kk == 0, stop=False)
            # bo broadcast-add: rank-1 ones^T x bo riding the accumulation
            mm(ps, ones_sb, bo_row[:, c * NC2:(c + 1) * NC2],
               start=False, stop=False)

        def e_last(sqt, c):
            mm(psE[(sqt, c)], ctxT[:, NT - 1, sqt * P:(sqt + 1) * P],
               wo[:, NT - 1, c * NC2:(c + 1) * NC2],
               start=False, stop=True)

        # flat (t, j) pipeline, scores emitted 2 steps ahead so neither PE
        # nor ACT bubbles at pair boundaries
        steps = [(t, j) for t in range(NPAIR) for j in range(SKT)]
        sps = {}

        def emit_scores(idx):
            if idx >= len(steps):
                return
            t, j = steps[idx]
            if j == 0 and t >= 1:
                drain_until([f"b{t}a", f"b{t}b"] +
                            ([f"c{t}"] if t >= 2 else []))
            sps[idx] = scores(t, j)

        pcx = {}
        rs = {}
        emit_scores(0)
        emit_scores(1)
        for idx, (t, j) in enumerate(steps):
            ep = epool.tile([P, 2, SQ], F16, name="ep")
            nc.scalar.activation(ep, sps.pop(idx), AF.Exp)
            emit_scores(idx + 2)
            drain_until([f"a{j}c{t // 4}"])
            if j % 2 == 1:
                drain_next(1)
            if t == NPAIR - 1 and j in (3, 5):
                # pair 7's only legal fillers: output-proj pre-accumulation
                ps = pp.tile([P, NC2], F32, name="ppt")
                e_pre(0, 0 if j == 3 else 1, ps)
            if j == 0:
                pcx[t] = (
                    pX.tile([HD + 1, SQ], F32, name="pcx0"),
                    pX.tile([HD + 1, SQ], F32, name="pcx1"),
                )
            pcx0, pcx1 = pcx[t]
            mm(pcx0, vh[:, j, 2 * t, :], ep[:, 0, :],
               start=j == 0, stop=j == SKT - 1)
            mm(pcx1, vh[:, j, 2 * t + 1, :], ep[:, 1, :],
               start=j == 0, stop=j == SKT - 1)
            if j == SKT - 1:
                # fast PSUM drain: sum-row copies + approx reciprocals gate
                # the (deferred) normalize, so they go first on DVE
                se0 = rpool.tile([1, SQ], F32, name="se0")
                se1 = rpool.tile([1, SQ], F32, name="se1")
                nc.vector.tensor_copy(se0, pcx0[HD:HD + 1, :])
                nc.vector.tensor_copy(se1, pcx1[HD:HD + 1, :])
                r0 = rpool.tile([1, SQ], F32, name="r0")
                r1 = rpool.tile([1, SQ], F32, name="r1")
                nc.vector.reciprocal_approx_fast(r0, se0)
                nc.vector.reciprocal_approx_fast(r1, se1)
                rs[t] = (r0, r1)
                if t == NPAIR - 1:
                    # split the last pair's ctx drain across ACT + DVE
                    nc.scalar.activation(ctxU[0:HD, t, :], pcx0[0:HD, :],
                                         AF.Copy)
                    nc.vector.tensor_copy(ctxU[HD:P, t, :], pcx1[0:HD, :])
                else:
                    nc.vector.tensor_copy(ctxU[0:HD, t, :], pcx0[0:HD, :])
                    nc.vector.tensor_copy(ctxU[HD:P, t, :], pcx1[0:HD, :])
            if j == 2 and t >= 1:
                normalize(t - 1, *rs.pop(t - 1))

        # ---- end phase ----
        # pre-accumulate the remaining six output groups on the PSUM banks
        # freed by the scores pool (4 banks) and ctx pool (2 banks); the PE
        # stays dense (~48 matmuls) while the pair-7 normalize chain runs on
        # gpsimd/DVE, so its latency is fully hidden
        # (reuse the "sp"/"pcx" slot names so the pools' banks are shared)
        # the 8 output half-DMAs rotate over the 3 queues so no queue ever
        # carries two halves of the same row-block back-to-back — the last
        # block's halves land on queues that drained earlier blocks
        eng_h0 = [nc.sync, nc.scalar, nc.gpsimd, nc.sync]
        eng_h1 = [nc.gpsimd, nc.sync, nc.scalar, nc.gpsimd]

        def e_finish(sqt):  # kk=7 + drains (ACT||DVE) + per-half DMAs
            e_last(sqt, 0)
            e_last(sqt, 1)
            o_sb = opool.tile([P, D], F16, name="o_sb")
            nc.scalar.activation(o_sb[:, 0:NC2], psE[(sqt, 0)], AF.Copy)
            eng_h0[sqt].dma_start(
                out_d[sqt * P:(sqt + 1) * P, 0:NC2], o_sb[:, 0:NC2]
            )
            nc.vector.tensor_copy(o_sb[:, NC2:D], psE[(sqt, 1)])
            eng_h1[sqt].dma_start(
                out_d[sqt * P:(sqt + 1) * P, NC2:D], o_sb[:, NC2:D]
            )

        psE_A = pS.tile([P, 2, NC2], F32, name="sp")
        e_pre(1, 0, psE_A[:, 0, :])
        e_pre(1, 1, psE_A[:, 1, :])
        psE_B = pS.tile([P, 2, NC2], F32, name="sp")
        e_pre(2, 0, psE_B[:, 0, :])
        normalize(NPAIR - 1, *rs.pop(NPAIR - 1))
        e_pre(2, 1, psE_B[:, 1, :])
        e_finish(0)
        psE_C = pX.tile([P, NC2], F32, name="pcx0")
        e_pre(3, 0, psE_C)
        e_finish(1)
        psE_D = pX.tile([P, NC2], F32, name="pcx1")
        e_pre(3, 1, psE_D)
        e_finish(2)
        e_finish(3)

    nc.compile()
    return nc


def get_program():
    if "nc" not in _CACHE:
        _CACHE["nc"] = _build_program()
    return _CACHE["nc"]


def make_in_maps(q, k, v, Wq, bq, Wk, bk, Wv, bv, Wo, bo):
    f32 = lambda x: np.ascontiguousarray(np.asarray(x, dtype=np.float32))
    blk = lambda wT: np.ascontiguousarray(
        np.asarray(wT, np.float16).reshape(NT, P, NT, P).transpose(2, 1, 0, 3)
    )
    # partition-major [p, kk, w]: per-partition data is one contiguous run,
    # so each DMA descriptor covers a full 16KB row (8x fewer descriptors)
    pmaj = lambda xT: np.ascontiguousarray(
        np.asarray(xT, np.float16).reshape(NT, P, -1).transpose(1, 0, 2)
    )
    q, k, v = np.asarray(q, np.float32), np.asarray(k, np.float32), \
        np.asarray(v, np.float32)
    wqT = blk(np.asarray(Wq, np.float32).T * np.float32(SCALE))
    wkT = blk(np.asarray(Wk, np.float32).T)
    wvT_pm = pmaj(np.asarray(Wv, np.float32).T)       # [P, NT, D]
    wvT = np.ascontiguousarray(
        wvT_pm.reshape(P, NT, 2, NC2).transpose(2, 0, 1, 3)
    )                                                 # [2, P, NT, NC2]
    woT = pmaj(np.asarray(Wo, np.float32).T)
    bqs = f32(bq) * np.float32(SCALE)
    # bv folds exactly through the output projection: softmax rows sum to 1,
    # so ctx gains +bv per head, and out gains +Wo@bv
    bo_ = (f32(bo) + np.asarray(Wo, np.float32) @ f32(bv)).astype(np.float16)
    # bk is exactly irrelevant: it shifts every score in a query row equally.
    # kT blocked by key-half, vT by key-block (first-use DMA granularity)
    kblk = lambda x: np.ascontiguousarray(
        pmaj(x).reshape(P, NT, 2, NC2).transpose(2, 0, 1, 3)
    )
    vblk = lambda x: np.ascontiguousarray(
        pmaj(x).reshape(P, NT, SKT, P).transpose(2, 0, 1, 3)
    )
    kTs = [kblk(k[b].T) for b in range(B)]
    vTs = [vblk(v[b].T) for b in range(B)]
    in_maps = []
    for core in range(N_CORES):
        b, half = divmod(core, 2)
        qT_c = pmaj(q[b, half * SQ:(half + 1) * SQ, :].T)
        in_maps.append({
            "qT": qT_c, "kT": kTs[b], "vT": vTs[b],
            "wqT": wqT, "wkT": wkT, "wvT": wvT, "woT": woT,
            "bq": bqs, "bo": bo_,
        })
    return in_maps


def gather_out(results):
    out = np.empty((B, S, D), dtype=np.float32)
    for core in range(N_CORES):
        b, half = divmod(core, 2)
        out[b, half * SQ:(half + 1) * SQ, :] = \
            results[core]["out"].astype(np.float32)
    return out


def kernel(q, k, v, Wq, bq, Wk, bk, Wv, bv, Wo, bo):
    from concourse.bass_utils import run_bass_kernel_spmd

    nc = get_program()
    in_maps = make_in_maps(q, k, v, Wq, bq, Wk, bk, Wv, bv, Wo, bo)
    res = run_bass_kernel_spmd(nc, in_maps, list(range(N_CORES)))
    return gather_out(res.results)


# revision 46
# speedup vs baseline: 1.0034x; 1.0034x over previous
"""Trainium2 Bass kernel for nn_Attention (B=4, S=1024, D=1024, H=16).

Sharding: 8 cores = 4 batches x 2 query-halves. Core i handles batch i//2,
query rows [(i%2)*512, (i%2)*512+512). Each core computes the full K/V
projections for its batch (duplicated across the 2 cores sharing a batch),
all 16 heads of attention for its query slice, and the output projection.
No collectives; the output is gathered host-side.

Device dataflow (per core) — fp16 matmul operands, fp32 PSUM accumulation:
  - host passes pre-transposed qT [D,SQ], kT/vT [D,SK], W{v,o}.T [D,D], and
    m-blocked W{k,q}.T [NT,D,128] (PE contracts over the partition dim, so
    both matmul operands need the contraction dim on partitions; transposing
    and blocking on host costs nothing on HW)
  - qhT[o,sq] = (Wq.T*SCALE).T-tiles @ qT     (o on partitions)
  - khT[o,sk] likewise; vh[sk, h, dh] natural via vT-as-stationary
  - scoresT[sk,sq] per head = khT-tile.T @ qhT; the two heads of a pair run
    as K=64 matmuls packed at PE row strips 0:64 / 64:128 (concurrent on
    disjoint row groups), writing the two halves of one [128, 2*SQ] PSUM
    tile -> ONE fused exp per pair
  - expT = exp(scoresT) on ACT (no max subtraction: |scores| < ~4 here,
    and softmax(x) == softmax(x - max) exactly)
  - ctxT_aug[dh+1, sq] += [vh | 1].T @ expT  (ones column makes row 64 the
    softmax denominator, riding free on the ctx matmul)
  - ctx PSUM is drained fast on DVE (sum-row copy + approx-reciprocal
    first, ctx rows after); the gpsimd broadcast + normalization multiply
    run later, off the critical path (the last pair broadcasts via a fp16
    ones-row matmul on the PE instead, so the output projection can start)
  - out[sq,o] = ctxT-tiles.T @ Wo.T (+bo via a K=1 ones-row matmul) in fp16

Scheduling (the trace-driven part):
  - input DMAs are spread across 4 engine queues in first-use order so the
    ramp is limited by HBM bandwidth, not queue serialization
  - the PE warms on c-groups (q-projection, gated only on qT+wq) instead of
    dummy matmuls during the DMA ramp
  - projection "filler" groups are drained into the attention loop with
    TARGETED just-in-time drains plus a slow generic pace (1 group per 2
    steps), so filler work survives into pairs 5-6 where exp latency would
    otherwise stall the PE (and let HAM re-throttle the clock)
  - during pair 7 the first two output-projection groups pre-accumulate
    kk=0..6; after the last ctx matmul the remaining six groups
    pre-accumulate on the PSUM banks just freed by the scores/ctx pools, so
    only eight kk=7 matmuls + drains remain after the final normalize
  - output is fp16 (host casts back), one [128,1024] DMA per row-block on
    rotating queues

Bias handling (exact): bq via ACT Identity-bias on the qh drain; bk dropped
(softmax is invariant to per-query score shifts); bv folded into bo on the
host (softmax rows sum to 1, so ctx gains +bv and out gains +Wo@bv); bo via
a K=1 ones-row matmul accumulated into the output-projection PSUM.
"""

import sys

import numpy as np

if "/opt/trn_rl_repo" not in sys.path:
    sys.path.insert(0, "/opt/trn_rl_repo")

B, S, D, H = 4, 1024, 1024, 16
HD = D // H                      # 64
SCALE = 1.0 / float(np.sqrt(HD))
N_CORES = 8
SQ = S // 2                      # 512 query rows per core
SK = S                           # full key length
P = 128
NT = D // P                      # 8 feature tiles
SKT = SK // P                    # 8 key tiles
NPAIR = H // 2                   # 8 head pairs
NC2 = 512                        # max matmul free dim (one PSUM bank)

_CACHE = {}


def _build_program():
    from contextlib import ExitStack

    import concourse.bass as bass
    import concourse.tile as tile
    from concourse import bacc, mybir

    F32 = mybir.dt.float32
    F16 = mybir.dt.float16
    AF = mybir.ActivationFunctionType

    nc = bacc.Bacc(
        "TRN2", target_bir_lowering=False, debug=False, num_devices=N_CORES
    )

    qT_d = nc.dram_tensor("qT", [P, NT, SQ], F16, kind="ExternalInput").ap()
    # kT blocked by key-half (b-group reads one half of every kk chunk);
    # vT blocked by key-block j (a-group reads one j-slice of every kk chunk)
    kT_d = nc.dram_tensor("kT", [2, P, NT, NC2], F16,
                          kind="ExternalInput").ap()
    vT_d = nc.dram_tensor("vT", [SKT, P, NT, P], F16,
                          kind="ExternalInput").ap()
    wqT_d = nc.dram_tensor("wqT", [NT, P, NT, P], F16,
                           kind="ExternalInput").ap()
    wkT_d = nc.dram_tensor("wkT", [NT, P, NT, P], F16,
                           kind="ExternalInput").ap()
    wvT_d = nc.dram_tensor("wvT", [2, P, NT, NC2], F16,
                           kind="ExternalInput").ap()
    woT_d = nc.dram_tensor("woT", [P, NT, D], F16, kind="ExternalInput").ap()
    bq_d = nc.dram_tensor("bq", [D], F32, kind="ExternalInput").ap()
    bo_d = nc.dram_tensor("bo", [D], F16, kind="ExternalInput").ap()
    out_d = nc.dram_tensor("out", [SQ, D], F16, kind="ExternalOutput").ap()

    mm = lambda *a, **k: nc.tensor.matmul(*a, **k)

    with tile.TileContext(nc) as tc, ExitStack() as ctx:
        persist = ctx.enter_context(tc.tile_pool(name="persist", bufs=1))
        epool = ctx.enter_context(tc.tile_pool(name="epool", bufs=4))
        rpool = ctx.enter_context(tc.tile_pool(name="rp", bufs=2))
        opool = ctx.enter_context(tc.tile_pool(name="outp", bufs=2))
        pp = ctx.enter_context(tc.tile_pool(name="pp", space="PSUM", bufs=2))
        pS = ctx.enter_context(tc.tile_pool(name="pS", space="PSUM", bufs=2))
        pX = ctx.enter_context(tc.tile_pool(name="pX", space="PSUM", bufs=1))

        # persistent data tiles
        qT_sb = persist.tile([P, NT, SQ], F16)
        kT_sb = persist.tile([P, NT, SK], F16)
        vT_sb = persist.tile([P, NT, SK], F16)
        wq = persist.tile([P, NT, D], F16)
        wk = persist.tile([P, NT, D], F16)
        wv = persist.tile([P, NT, D], F16)
        wo = persist.tile([P, NT, D], F16)
        qhT = persist.tile([P, NT, SQ], F16)        # [o'%128, o'//128, sq]
        khT = persist.tile([P, NT, SK], F16)
        vh = persist.tile([P, SKT, H, HD + 1], F16)  # [sk%128, sk//128, h, .]
        ctxT = persist.tile([P, NT, SQ], F16)
        ctxU = persist.tile([P, NT, SQ], F16)    # unnormalized ctx (drain)
        bq_sb = persist.tile([P, NT], F32)
        bo_row = persist.tile([1, D], F16)

        # ---- input DMAs: first-use priority, spread over 3 queues ----
        # descriptor generation costs ~1.5-3us of engine time per dma, so
        # each queue's issue order is also its engine-time budget: the
        # scalar queue carries only what gates pair 0, sync carries the
        # k-side, gpsimd the v-side
        # per-kk chunks so subtile deps release each contraction matmul as
        # its chunk lands, instead of gating pair 0 on whole-tensor DMAs
        nc.scalar.dma_start(qT_sb[:, 0, :], qT_d[:, 0, :])
        nc.scalar.dma_start(wq[:, :, 0:P], wqT_d[0])
        for kk in range(1, NT):
            nc.scalar.dma_start(qT_sb[:, kk, :], qT_d[:, kk, :])
        nc.scalar.dma_start(wq[:, :, P:2 * P], wqT_d[1])
        nc.scalar.dma_start(wv[:, :, 0:NC2], wvT_d[0])
        nc.scalar.dma_start(wq[:, :, 2 * P:3 * P], wqT_d[2])
        nc.scalar.dma_start(wq[:, :, 3 * P:4 * P], wqT_d[3])
        nc.sync.dma_start(kT_sb[:, :, 0:NC2], kT_d[0])
        nc.sync.dma_start(wk[:, :, 0:P], wkT_d[0])
        nc.sync.dma_start(kT_sb[:, :, NC2:SK], kT_d[1])
        for m in range(1, NT):
            nc.sync.dma_start(wk[:, :, m * P:(m + 1) * P], wkT_d[m])
        nc.gpsimd.dma_start(out=bq_sb, in_=bq_d.rearrange("(m p) -> p m", p=P))
        nc.gpsimd.dma_start(out=bo_row, in_=bo_d.rearrange("(o d) -> o d", o=1))
        for j in range(SKT):
            nc.gpsimd.dma_start(vT_sb[:, :, j * P:(j + 1) * P], vT_d[j])
        for m in range(4, NT):
            nc.sync.dma_start(wq[:, :, m * P:(m + 1) * P], wqT_d[m])
        nc.gpsimd.dma_start(wv[:, :, NC2:D], wvT_d[1])
        nc.sync.dma_start(wo, woT_d)
        for j in range(SKT):
            nc.vector.memset(vh[:, j, :, HD].bitcast(mybir.dt.uint16), 0x3C00)
        warm = rpool.tile([1, 1], F32, name="warm")
        nc.vector.memset(warm, 0.0)
        nc.scalar.activation(warm, warm, AF.Exp)
        # short dummy-matmul ramp on a zeroed tile: HAM sees a busy PE and
        # unthrottles to full clock just as the first real matmuls start
        wz = persist.tile([P, NC2], F16)
        nc.vector.memset(wz, 0.0)

        def pe_warm(n):
            psw = pp.tile([P, NC2], F32, name="ppt")
            for _ in range(n):
                mm(psw, wz[:, 0:P], wz, start=True, stop=True)

        pe_warm(12)
        ones_sb = persist.tile([1, P], F16)
        nc.vector.memset(ones_sb, 1.0)

        # ---- emit-group helpers (each = one 8-MM PSUM accumulation) ----
        def a_group(j, c):  # v-proj: vh[:, j, heads c*8..c*8+7]
            psa = pp.tile([P, NC2], F32, name="ppt")
            for kk in range(NT):
                mm(psa, vT_sb[:, kk, j * P:(j + 1) * P],
                   wv[:, kk, c * NC2:(c + 1) * NC2],
                   start=kk == 0, stop=kk == NT - 1)
            nc.vector.tensor_copy(
                vh[:, j, c * 8:(c + 1) * 8, 0:HD],
                psa.rearrange("p (h d) -> p h d", d=HD),
            )

        def b_group(m, c):  # k-proj: khT[:, m, c*512:...]
            psb = pp.tile([P, NC2], F32, name="ppt")
            for kk in range(NT):
                mm(psb, wk[:, kk, m * P:(m + 1) * P],
                   kT_sb[:, kk, c * NC2:(c + 1) * NC2],
                   start=kk == 0, stop=kk == NT - 1)
            nc.vector.tensor_copy(khT[:, m, c * NC2:(c + 1) * NC2], psb)

        def c_group(m):  # q-proj: qhT[:, m, :]
            psc = pp.tile([P, NC2], F32, name="ppt")
            for kk in range(NT):
                mm(psc, wq[:, kk, m * P:(m + 1) * P], qT_sb[:, kk, :],
                   start=kk == 0, stop=kk == NT - 1)
            nc.vector.tensor_scalar_add(qhT[:, m, :], psc, bq_sb[:, m:m + 1])

        # ---- ramp: only what pair 0 needs, so scores start ~25us ----
        c_group(0)
        b_group(0, 0)
        b_group(0, 1)
        c_group(1)

        # ---- filler stream with targeted + paced drains ----
        filler = []          # ordered list of (label, emit_fn)
        emitted = set()
        by_label = {}

        def drain_until(labels):
            # targeted: emit exactly the named groups (in the given order);
            # labels not in the filler set (e.g. ramp-emitted c0/c1) skip
            for lbl in labels:
                if lbl in by_label and lbl not in emitted:
                    emitted.add(lbl)
                    by_label[lbl]()

        def drain_next(n=1):
            done = 0
            for lbl, fn in filler:
                if lbl not in emitted:
                    emitted.add(lbl)
                    fn()
                    done += 1
                    if done >= n:
                        return

        def add_filler(lbl, fn):
            filler.append((lbl, fn))
            by_label[lbl] = fn

        add_filler("c2", lambda: c_group(2))
        add_filler("c3", lambda: c_group(3))
        add_filler("c4", lambda: c_group(4))
        add_filler("c5", lambda: c_group(5))
        for j in range(SKT):
            add_filler(f"a{j}c0", lambda j=j: a_group(j, 0))
        for m in (1, 2, 3):
            add_filler(f"b{m}a", lambda m=m: b_group(m, 0))
            add_filler(f"b{m}b", lambda m=m: b_group(m, 1))
        for j in range(4):
            add_filler(f"a{j}c1", lambda j=j: a_group(j, 1))
        add_filler("b4a", lambda: b_group(4, 0))
        add_filler("b4b", lambda: b_group(4, 1))
        for j in range(4, SKT):
            add_filler(f"a{j}c1", lambda j=j: a_group(j, 1))
        add_filler("b5a", lambda: b_group(5, 0))
        add_filler("b5b", lambda: b_group(5, 1))
        add_filler("c6", lambda: c_group(6))
        add_filler("b6a", lambda: b_group(6, 0))
        add_filler("b6b", lambda: b_group(6, 1))
        add_filler("c7", lambda: c_group(7))
        add_filler("b7a", lambda: b_group(7, 0))
        add_filler("b7b", lambda: b_group(7, 1))

        # ---- attention ----
        def scores(t, j):
            sp = pS.tile([P, 2, SQ], F32, name="sp")
            mm(sp[:, 0, :], khT[0:HD, t, j * P:(j + 1) * P], qhT[0:HD, t, :],
               start=True, stop=True)
            mm(sp[:, 1, :], khT[HD:P, t, j * P:(j + 1) * P], qhT[HD:P, t, :],
               start=True, stop=True)
            return sp

        def normalize(t, r0, r1):
            rb0 = rpool.tile([P, SQ], F32, name="rb0")
            rb1 = rpool.tile([P, SQ], F32, name="rb1")
            nc.gpsimd.partition_broadcast(rb0, r0)
            nc.gpsimd.partition_broadcast(rb1, r1)
            nc.vector.tensor_mul(ctxT[0:HD, t, :], ctxU[0:HD, t, :],
                                 rb0[0:HD, :])
            nc.vector.tensor_mul(ctxT[HD:P, t, :], ctxU[HD:P, t, :],
                                 rb1[HD:P, :])

        # output-projection groups: G[sqt][c], pre-accumulated kk=0..6 (+bo)
        # then finished with the kk=7 matmul once pair 7's ctxT lands
        psE = {}

        def e_pre(sqt, c, ps):
            psE[(sqt, c)] = ps
            for kk in range(NT - 1):
                mm(ps, ctxT[:, kk, sqt * P:(sqt + 1) * P],
                   wo[:, kk, c * NC2:(c + 1) * NC2],
                   start=kk == 0, stop=False)
            # bo broadcast-add: rank-1 ones^T x bo riding the accumulation
            mm(ps, ones_sb, bo_row[:, c * NC2:(c + 1) * NC2],
               start=False, stop=False)

        def e_last(sqt, c):
            mm(psE[(sqt, c)], ctxT[:, NT - 1, sqt * P:(sqt + 1) * P],
               wo[:, NT - 1, c * NC2:(c + 1) * NC2],
               start=False, stop=True)

        # flat (t, j) pipeline, scores emitted 2 steps ahead so neither PE
        # nor ACT bubbles at pair boundaries
        steps = [(t, j) for t in range(NPAIR) for j in range(SKT)]
        sps = {}

        def emit_scores(idx):
            if idx >= len(steps):
                return
            t, j = steps[idx]
            if j == 0 and t >= 1:
                drain_until([f"b{t}a", f"b{t}b"] +
                            ([f"c{t}"] if t >= 2 else []))
            sps[idx] = scores(t, j)

        pcx = {}
        rs = {}
        emit_scores(0)
        emit_scores(1)
        for idx, (t, j) in enumerate(steps):
            ep = epool.tile([P, 2, SQ], F16, name="ep")
            nc.scalar.activation(ep, sps.pop(idx), AF.Exp)
            emit_scores(idx + 2)
            drain_until([f"a{j}c{t // 4}"])
            if j % 2 == 1:
                drain_next(1)
            if t == NPAIR - 1 and j in (3, 5):
                # pair 7's only legal fillers: output-proj pre-accumulation
                ps = pp.tile([P, NC2], F32, name="ppt")
                e_pre(0, 0 if j == 3 else 1, ps)
            if j == 0:
                pcx[t] = (
                    pX.tile([HD + 1, SQ], F32, name="pcx0"),
                    pX.tile([HD + 1, SQ], F32, name="pcx1"),
                )
            pcx0, pcx1 = pcx[t]
            mm(pcx0, vh[:, j, 2 * t, :], ep[:, 0, :],
               start=j == 0, stop=j == SKT - 1)
            mm(pcx1, vh[:, j, 2 * t + 1, :], ep[:, 1, :],
               start=j == 0, stop=j == SKT - 1)
            if j == SKT - 1:
                # fast PSUM drain: sum-row copies + approx reciprocals gate
                # the (deferred) normalize, so they go first on DVE
                se0 = rpool.tile([1, SQ], F32, name="se0")
                se1 = rpool.tile([1, SQ], F32, name="se1")
                nc.vector.tensor_copy(se0, pcx0[HD:HD + 1, :])
                nc.vector.tensor_copy(se1, pcx1[HD:HD + 1, :])
                r0 = rpool.tile([1, SQ], F32, name="r0")
                r1 = rpool.tile([1, SQ], F32, name="r1")
                nc.vector.reciprocal_approx_fast(r0, se0)
                nc.vector.reciprocal_approx_fast(r1, se1)
                rs[t] = (r0, r1)
                if t == NPAIR - 1:
                    # split the last pair's ctx drain across ACT + DVE
                    nc.scalar.activation(ctxU[0:HD, t, :], pcx0[0:HD, :],
                                         AF.Copy)
                    nc.vector.tensor_copy(ctxU[HD:P, t, :], pcx1[0:HD, :])
                else:
                    nc.vector.tensor_copy(ctxU[0:HD, t, :], pcx0[0:HD, :])
                    nc.vector.tensor_copy(ctxU[HD:P, t, :], pcx1[0:HD, :])
            if j == 2 and t >= 1:
                normalize(t - 1, *rs.pop(t - 1))

        # ---- end phase ----
        # pre-accumulate the remaining six output groups on the PSUM banks
        # freed by the scores pool (4 banks) and ctx pool (2 banks); the PE
        # stays dense (~48 matmuls) while the pair-7 normalize chain runs on
        # gpsimd/DVE, so its latency is fully hidden
        # (reuse the "sp"/"pcx" slot names so the pools' banks are shared)
        # one full-row DMA per block: 2KB contiguous DRAM lines drain ~3x
        # faster than per-half 1KB lines, and the queues rotate so the last
        # block lands on a queue that finished its earlier block long ago
        dma_engines = [nc.sync, nc.gpsimd, nc.scalar, nc.sync]

        def e_finish(sqt):  # kk=7 + drains (ACT||DVE) + one 2KB-line DMA
            e_last(sqt, 0)
            e_last(sqt, 1)
            o_sb = opool.tile([P, D], F16, name="o_sb")
            nc.scalar.activation(o_sb[:, 0:NC2], psE[(sqt, 0)], AF.Copy)
            nc.vector.tensor_copy(o_sb[:, NC2:D], psE[(sqt, 1)])
            dma_engines[sqt].dma_start(
                out_d[sqt * P:(sqt + 1) * P, :], o_sb
            )

        psE_A = pS.tile([P, 2, NC2], F32, name="sp")
        e_pre(1, 0, psE_A[:, 0, :])
        e_pre(1, 1, psE_A[:, 1, :])
        psE_B = pS.tile([P, 2, NC2], F32, name="sp")
        e_pre(2, 0, psE_B[:, 0, :])
        normalize(NPAIR - 1, *rs.pop(NPAIR - 1))
        e_pre(2, 1, psE_B[:, 1, :])
        e_finish(0)
        psE_C = pX.tile([P, NC2], F32, name="pcx0")
        e_pre(3, 0, psE_C)
        e_finish(1)
        psE_D = pX.tile([P, NC2], F32, name="pcx1")
        e_pre(3, 1, psE_D)
        e_finish(2)
        e_finish(3)

    nc.compile()
    return nc


def get_program():
    if "nc" not in _CACHE:
        _CACHE["nc"] = _build_program()
    return _CACHE["nc"]


def make_in_maps(q, k, v, Wq, bq, Wk, bk, Wv, bv, Wo, bo):
    f32 = lambda x: np.ascontiguousarray(np.asarray(x, dtype=np.float32))
    blk = lambda wT: np.ascontiguousarray(
        np.asarray(wT, np.float16).reshape(NT, P, NT, P).transpose(2, 1, 0, 3)
    )
    # partition-major [p, kk, w]: per-partition data is one contiguous run,
    # so each DMA descriptor covers a full 16KB row (8x fewer descriptors)
    pmaj = lambda xT: np.ascontiguousarray(
        np.asarray(xT, np.float16).reshape(NT, P, -1).transpose(1, 0, 2)
    )
    q, k, v = np.asarray(q, np.float32), np.asarray(k, np.float32), \
        np.asarray(v, np.float32)
    wqT = blk(np.asarray(Wq, np.float32).T * np.float32(SCALE))
    wkT = blk(np.asarray(Wk, np.float32).T)
    wvT_pm = pmaj(np.asarray(Wv, np.float32).T)       # [P, NT, D]
    wvT = np.ascontiguousarray(
        wvT_pm.reshape(P, NT, 2, NC2).transpose(2, 0, 1, 3)
    )                                                 # [2, P, NT, NC2]
    woT = pmaj(np.asarray(Wo, np.float32).T)
    bqs = f32(bq) * np.float32(SCALE)
    # bv folds exactly through the output projection: softmax rows sum to 1,
    # so ctx gains +bv per head, and out gains +Wo@bv
    bo_ = (f32(bo) + np.asarray(Wo, np.float32) @ f32(bv)).astype(np.float16)
    # bk is exactly irrelevant: it shifts every score in a query row equally.
    # kT blocked by key-half, vT by key-block (first-use DMA granularity)
    kblk = lambda x: np.ascontiguousarray(
        pmaj(x).reshape(P, NT, 2, NC2).transpose(2, 0, 1, 3)
    )
    vblk = lambda x: np.ascontiguousarray(
        pmaj(x).reshape(P, NT, SKT, P).transpose(2, 0, 1, 3)
    )
    kTs = [kblk(k[b].T) for b in range(B)]
    vTs = [vblk(v[b].T) for b in range(B)]
    in_maps = []
    for core in range(N_CORES):
        b, half = divmod(core, 2)
        qT_c = pmaj(q[b, half * SQ:(half + 1) * SQ, :].T)
        in_maps.append({
            "qT": qT_c, "kT": kTs[b], "vT": vTs[b],
            "wqT": wqT, "wkT": wkT, "wvT": wvT, "woT": woT,
            "bq": bqs, "bo": bo_,
        })
    return in_maps


def gather_out(results):
    out = np.empty((B, S, D), dtype=np.float32)
    for core in range(N_CORES):
        b, half = divmod(core, 2)
        out[b, half * SQ:(half + 1) * SQ, :] = \
            results[core]["out"].astype(np.float32)
    return out


def kernel(q, k, v, Wq, bq, Wk, bk, Wv, bv, Wo, bo):
    from concourse.bass_utils import run_bass_kernel_spmd

    nc = get_program()
    in_maps = make_in_maps(q, k, v, Wq, bq, Wk, bk, Wv, bv, Wo, bo)
    res = run_bass_kernel_spmd(nc, in_maps, list(range(N_CORES)))
    return gather_out(res.results)
